# revision 42
# baseline (speedup 1.0000x reference)
"""Trainium2 Bass kernel for nn_BaseMPNN (GNN message passing), 8-core SPMD.

Design (vs the v2 pair-packed baseline):
- Gather: two plain-mode dma_gathers per 4-window group over single 256B
  node rows (lo: htab[0:32768] idx=row; hi: htab[18432:] idx=row-18432).
  Edges whose source row falls in the overlap [18432,32768) go to whichever
  section balances both to exactly 2 chunks of 128. No pair packing, no
  parity-select DVE chain, half the gather bytes.
- Nodes are LPT-balanced across cores/windows (assignment is pure data:
  onehot / index metadata), so every window holds <=512 edges in 4 chunks
  (2 lo + 2 hi) instead of 5 -> ~20% less edge work, ~0.35% slot padding.
- onehot (scatter rhs, carries rdeg so no separate scale) and eT (edge
  features, updated in place) are SBUF-resident all 3 layers; ohne streams.
- h table, h state, and all matmuls are bf16 (folded BN weights cast).
- Stats accumulate during the sweep via activation accum_out plus pad
  correction chains (epad for e, hpad for pad nodes), so no end-of-layer
  stats matmul tail.
"""

import math
from contextlib import ExitStack
from dataclasses import dataclass

import numpy as np
import ml_dtypes

import concourse.bacc as bacc
import concourse.bass as bass
import concourse.tile as tile
from concourse import mybir
from concourse.masks import make_identity

F32 = mybir.dt.float32
BF16 = mybir.dt.bfloat16
I16 = mybir.dt.int16
P = 128
EPS = 1e-5
BF = ml_dtypes.bfloat16

LO_LIM = 32768        # lo gather covers table rows [0, 32768)
HI_BASE = 18432       # hi gather covers table rows [18432, NPADT)


@dataclass(frozen=True)
class Cfg:
    NC: int = 8        # cores
    H: int = 128       # hidden (must be 128)
    F: int = 16        # input features
    L: int = 3         # meta layers
    NW: int = 49       # 128-node windows per core
    CLO: int = 2       # lo chunks per window
    CHI: int = 2       # hi chunks per window
    GW: int = 4        # windows per gather group
    N_real: int = 50000
    E_real: int = 200000

    @property
    def CPW(self):
        return self.CLO + self.CHI

    @property
    def NPC(self):   # compute nodes per core
        return self.NW * P

    @property
    def NPCT(self):  # node-table rows per core (incl. zero pad window)
        return (self.NW + 1) * P

    @property
    def NPADT(self):  # global node-table rows
        return self.NC * self.NPCT

    @property
    def ECAP(self):  # edge slots per core
        return self.NW * self.CPW * P

    @property
    def CHUNKS(self):
        return self.NW * self.CPW

    @property
    def NG(self):    # gather groups
        return math.ceil(self.NW / self.GW)

    @property
    def ZLO(self):   # guaranteed-zero table row for lo-section pads
        return self.NPC

    @property
    def ZHI(self):   # guaranteed-zero table row for hi-section pads
        return (self.NC - 1) * self.NPCT + self.NPC


def _wrap16(flat):
    """int16 flat index list -> [128, n/16] wrap-16, replicated x8 groups."""
    n = len(flat)
    assert n % 16 == 0
    w = flat.reshape(n // 16, 16).T
    return np.ascontiguousarray(np.tile(w, (8, 1)))


def _balance_nodes(cfg: Cfg, col):
    """LPT-pack nodes into NC*NW windows (<=128 nodes, ~equal edge counts).

    Returns (node_wg, node_slot): window-global id and slot for each node.
    """
    import heapq

    NBINS = cfg.NC * cfg.NW
    deg = np.bincount(col, minlength=cfg.N_real)
    order = np.argsort(-deg, kind="stable")
    node_wg = np.empty(cfg.N_real, np.int64)
    node_slot = np.empty(cfg.N_real, np.int64)
    heap = [(0, b) for b in range(NBINS)]
    heapq.heapify(heap)
    nodes_in = np.zeros(NBINS, np.int64)
    for n in order:
        edges, b = heapq.heappop(heap)
        node_wg[n] = b
        node_slot[n] = nodes_in[b]
        nodes_in[b] += 1
        if nodes_in[b] < P:
            heapq.heappush(heap, (edges + int(deg[n]), b))
    return node_wg, node_slot


def prep(cfg: Cfg, x, edge_index, edge_attr):
    """Host-side preprocessing -> per-core input maps."""
    x = np.asarray(x, np.float32)
    ei = np.asarray(edge_index, np.int64)
    ea = np.asarray(edge_attr, np.float32)
    row, col = ei[0], ei[1]
    NPC, NW, CPW, CLO, ECAP, CHUNKS = (
        cfg.NPC, cfg.NW, cfg.CPW, cfg.CLO, cfg.ECAP, cfg.CHUNKS,
    )
    LOSL, HISL = cfg.CLO * P, cfg.CHI * P

    deg = np.bincount(col, minlength=cfg.N_real).astype(np.float32)
    rdeg_all = 1.0 / np.maximum(deg, 1.0)

    node_wg, node_slot = _balance_nodes(cfg, col)
    node_core = node_wg // NW
    node_wl = node_wg % NW
    tid = node_core * cfg.NPCT + node_wl * P + node_slot  # global table row

    e_wg = node_wg[col]
    e_order = np.argsort(e_wg, kind="stable")
    ewg_sorted = e_wg[e_order]
    wstart = np.searchsorted(ewg_sorted, np.arange(cfg.NC * NW))
    wend = np.searchsorted(ewg_sorted, np.arange(cfg.NC * NW) + 1)

    maps = []
    for c in range(cfg.NC):
        # per-slot metadata, linear slot index = chunk*128 + partition
        idx_lo = np.full(NW * LOSL, cfg.ZLO, np.int64)
        idx_hi = np.full(NW * HISL, cfg.ZHI - HI_BASE, np.int64)
        oh_n = np.full(ECAP, -1, np.int64)    # col node slot (-1 = pad)
        redge = np.zeros(ECAP, np.float32)
        ea_slots = np.zeros((ECAP, cfg.F), np.float32)

        nodes_c = int((node_core == np.int64(c)).sum())
        for w in range(NW):
            wg = c * NW + w
            sel = e_order[wstart[wg]:wend[wg]]
            erow_t = tid[row[sel]]
            is_lof = erow_t < HI_BASE
            is_hif = erow_t >= LO_LIM
            is_flex = ~is_lof & ~is_hif
            n_lof, n_hif = int(is_lof.sum()), int(is_hif.sum())
            n_flex = int(is_flex.sum())
            assert n_lof + n_hif + n_flex <= CPW * P, (c, w)
            assert n_lof <= LOSL and n_hif <= HISL, (c, w, n_lof, n_hif)
            lo_take = min(n_flex, LOSL - n_lof)
            assert n_hif + (n_flex - lo_take) <= HISL, (c, w)
            flex_idx = np.nonzero(is_flex)[0]
            lo_sel = np.concatenate([np.nonzero(is_lof)[0], flex_idx[:lo_take]])
            hi_sel = np.concatenate([np.nonzero(is_hif)[0], flex_idx[lo_take:]])

            for base_chunk, ssel, ibuf, ioff, rebase in (
                (0, lo_sel, idx_lo, w * LOSL, 0),
                (CLO, hi_sel, idx_hi, w * HISL, HI_BASE),
            ):
                cnt = len(ssel)
                eids = sel[ssel]
                ibuf[ioff:ioff + cnt] = erow_t[ssel] - rebase
                pos = np.arange(cnt)
                slot = (w * CPW + base_chunk + pos // P) * P + pos % P
                oh_n[slot] = node_slot[col[eids]]
                redge[slot] = rdeg_all[col[eids]]
                ea_slots[slot] = ea[eids]

        assert idx_lo.max() < LO_LIM and idx_lo.min() >= 0
        assert idx_hi.max() <= 32767 and idx_hi.min() >= 0

        # group-wrapped gather index buffers
        def wrap_groups(ibuf, secsl):
            parts = []
            for g in range(cfg.NG):
                w0 = g * cfg.GW
                gsz = min(cfg.GW, NW - w0)
                seg = ibuf[w0 * secsl:(w0 + gsz) * secsl].astype(np.int16)
                parts.append(_wrap16(seg))
            return np.concatenate(parts, axis=1)

        ipr_lo = wrap_groups(idx_lo, LOSL)
        ipr_hi = wrap_groups(idx_hi, HISL)

        # onehot [e-part, chunk*128 + node-slot] carries rdeg (t1w semantics,
        # so the scatter matmul needs no separate rdeg scale); ohne is 0/1.
        slots = np.arange(ECAP)
        p_arr, k_arr = slots % P, slots // P
        onehot = np.zeros((P, ECAP), BF)
        valid = oh_n >= 0
        onehot[p_arr[valid], k_arr[valid] * P + oh_n[valid]] = redge[valid]
        ohne = np.zeros((P, ECAP), BF)
        ohne[oh_n[valid], k_arr[valid] * P + p_arr[valid]] = 1

        csel = node_core == np.int64(c)
        xT = np.zeros((cfg.F, NPC), BF)
        xT[:, node_wl[csel] * P + node_slot[csel]] = x[csel].T
        maskw = np.zeros((P, NW), np.float32)
        maskw[node_slot[csel], node_wl[csel]] = 1.0

        n_edges = int(wend[c * NW + NW - 1] - wstart[c * NW])
        maps.append(
            {
                "npads": np.full((P, 1), float(ECAP - n_edges), np.float32),
                "npadn": np.full((P, 1), float(NPC - nodes_c), np.float32),
                "xT": np.ascontiguousarray(xT),
                "eaT": np.ascontiguousarray(ea_slots.T.astype(BF)),
                "ipr_lo": ipr_lo,
                "ipr_hi": ipr_hi,
                "onehot": np.ascontiguousarray(onehot),
                "ohne": np.ascontiguousarray(ohne),
                "maskw": maskw,
            }
        )
    return maps


def add_weight_params(cfg: Cfg, maps, w):
    """Append (replicated) weight arrays to each core's input map."""
    H, L = cfg.H, cfg.L

    def col(a):
        return np.asarray(a, np.float32).reshape(H, 1)

    shared = {
        "enc_node_w": np.asarray(w["enc_node_w"], np.float32).astype(BF),
        "enc_edge_w": np.asarray(w["enc_edge_w"], np.float32).astype(BF),
        "edge_w": np.asarray(w["edge_w"], np.float32),
        "edge_b_col": np.asarray(w["edge_b"], np.float32).reshape(L, H, 1),
        "n1_w": np.asarray(w["n1_w"], np.float32),
        "n1_b_col": np.asarray(w["n1_b"], np.float32).reshape(L, H, 1),
        "n2_w": np.asarray(w["n2_w"], np.float32),
        "n2_b_col": np.asarray(w["n2_b"], np.float32).reshape(L, H, 1),
        "bn_node_g": col(w["bn_node_g"]),
        "bn_node_b": col(w["bn_node_b"]),
        "bn_edge_g": col(w["bn_edge_g"]),
        "bn_edge_b": col(w["bn_edge_b"]),
        "reg_w": np.asarray(w["reg_w"], np.float32).reshape(2 * H, 1),
        "reg_b": np.asarray(w["reg_b"], np.float32).reshape(1, 1),
    }
    for k in ["enc_node_b", "enc_edge_b", "edge_b", "n1_b", "n2_b"]:
        assert np.all(np.asarray(w[k]) == 0.0), f"nonzero bias {k} unsupported"
    for m in maps:
        m.update(shared)
    return maps


def build(cfg: Cfg):
    """Build the SPMD Bass program. Returns nc."""
    H, F, L, NW, GW = cfg.H, cfg.F, cfg.L, cfg.NW, cfg.GW
    NPC, NPCT, NPADT, ECAP = cfg.NPC, cfg.NPCT, cfg.NPADT, cfg.ECAP
    NG, CHUNKS, CPW, CLO = cfg.NG, cfg.CHUNKS, cfg.CPW, cfg.CLO
    WSL = CPW * P       # 512 edge slots per window
    LOSL, HISL = cfg.CLO * P, cfg.CHI * P
    inv_n = 1.0 / cfg.N_real
    inv_e = 1.0 / cfg.E_real

    nc = bacc.Bacc(
        "TRN2", target_bir_lowering=False, debug=False, num_devices=cfg.NC,
        num_swdge_queues=4,
    )

    def param(name, shape, dt=F32):
        return nc.declare_dram_parameter(name, list(shape), dt, isOutput=False).ap()

    xT = param("xT", [F, NPC], BF16)
    eaT = param("eaT", [F, ECAP], BF16)
    ipr_lo_p = param("ipr_lo", [P, NW * LOSL // 16], I16)
    ipr_hi_p = param("ipr_hi", [P, NW * HISL // 16], I16)
    onehot_p = param("onehot", [P, ECAP], BF16)
    ohne_p = param("ohne", [P, ECAP], BF16)
    maskw_p = param("maskw", [P, NW])
    npads_p = param("npads", [P, 1])
    npadn_p = param("npadn", [P, 1])
    enc_node_w = param("enc_node_w", [F, H], BF16)
    enc_edge_w = param("enc_edge_w", [F, H], BF16)
    edge_w_p = param("edge_w", [L, 3 * H, H])
    edge_b_col_p = param("edge_b_col", [L, H, 1])
    n1_w_p = param("n1_w", [L, 2 * H, H])
    n1_b_col_p = param("n1_b_col", [L, H, 1])
    n2_w_p = param("n2_w", [L, 2 * H, H])
    n2_b_col_p = param("n2_b_col", [L, H, 1])
    bn_node_g = param("bn_node_g", [H, 1])
    bn_node_b = param("bn_node_b", [H, 1])
    bn_edge_g = param("bn_edge_g", [H, 1])
    bn_edge_b = param("bn_edge_b", [H, 1])
    reg_w_p = param("reg_w", [2 * H, 1])
    reg_b_p = param("reg_b", [1, 1])
    out_p = nc.declare_dram_parameter("out", [1, 1], F32, isOutput=True).ap()

    hseg = [nc.dram_tensor(f"hseg_{i}", [NPCT, H], BF16).ap() for i in range(L)]
    htab = [
        nc.dram_tensor(f"htab_{i}", [NPADT, H], BF16, addr_space="Shared").ap()
        for i in range(L)
    ]
    ar_in = [nc.dram_tensor(f"ar_in_{i}", [H, 4], F32).ap() for i in range(L)]
    ar_out = [
        nc.dram_tensor(f"ar_out_{i}", [H, 4], F32, addr_space="Shared").ap()
        for i in range(L)
    ]
    rg = [list(range(cfg.NC))]

    AluOp = mybir.AluOpType
    Act = mybir.ActivationFunctionType

    with tile.TileContext(nc) as tc, ExitStack() as ctx:
        singles = ctx.enter_context(tc.tile_pool(name="singles", bufs=1))
        spool = ctx.enter_context(tc.tile_pool(name="spool", bufs=1))
        wpool = ctx.enter_context(tc.tile_pool(name="wpool", bufs=3))
        cpool = ctx.enter_context(tc.tile_pool(name="cpool", bufs=4))
        gpool = ctx.enter_context(tc.tile_pool(name="gpool", bufs=3))
        # PSUM budget: 8 banks of [128, 2KB]. pse 2 + psT 2 + psm 2 + psa 1
        # + shared psf/mix 1.
        ps_e = ctx.enter_context(tc.tile_pool(name="ps_e", bufs=2, space="PSUM"))
        ps_t = ctx.enter_context(tc.tile_pool(name="ps_t", bufs=2, space="PSUM"))
        ps_m = ctx.enter_context(tc.tile_pool(name="ps_m", bufs=2, space="PSUM"))
        ps_a = ctx.enter_context(tc.tile_pool(name="ps_a", bufs=1, space="PSUM"))
        ps_f = ctx.enter_context(tc.tile_pool(name="ps_f", bufs=1, space="PSUM"))
        ps_x = ps_f

        ones_row = singles.tile([1, P], F32)
        nc.vector.memset(ones_row[:], 1.0)
        ident1 = singles.tile([1, 1], F32)
        nc.vector.memset(ident1[:], 1.0)
        ident_f = singles.tile([P, P], F32)
        make_identity(nc, ident_f[:])
        ident_bf = singles.tile([P, P], BF16)
        nc.vector.tensor_copy(out=ident_bf[:], in_=ident_f[:])
        eps_sb = singles.tile([P, 1], F32)
        nc.vector.memset(eps_sb[:], EPS)
        zero_bf = singles.tile([P, P], BF16)
        nc.vector.memset(zero_bf[:], 0.0)

        def load(name_, shape, src, dt=F32, pool=singles):
            t = pool.tile(shape, dt, tag=name_, name=name_)
            nc.sync.dma_start(out=t[:], in_=src)
            return t

        ipr_lo_sb = load("ipr_lo_sb", [P, NW * LOSL // 16], ipr_lo_p[:, :], I16)
        ipr_hi_sb = load("ipr_hi_sb", [P, NW * HISL // 16], ipr_hi_p[:, :], I16)
        # biggest param load (6.4MB) goes on the Scalar engine's DMA queue
        # so it doesn't serialize the sync queue ahead of the encoder loads
        onehot_sb = singles.tile([P, ECAP], BF16, tag="onehot_sb",
                                 name="onehot_sb")
        nc.scalar.dma_start(out=onehot_sb[:], in_=onehot_p[:, :])
        maskw_sb = load("maskw_sb", [P, NW], maskw_p[:, :])
        npads_sb = load("npads_sb", [P, 1], npads_p[:, :])
        npadn_sb = load("npadn_sb", [P, 1], npadn_p[:, :])
        encn_w = load("encn_w", [F, H], enc_node_w[:, :], BF16)
        ence_w = load("ence_w", [F, H], enc_edge_w[:, :], BF16)
        # one DMA per weight family (vs 21 small ones) via rearranged views
        we_all = load("we_all", [P, L, 3, H],
                      edge_w_p.rearrange("l (k p) h -> p l k h", p=P))
        wn1_all = load("wn1_all", [P, L, 2, H],
                       n1_w_p.rearrange("l (k p) h -> p l k h", p=P))
        wn2_all = load("wn2_all", [P, L, 2, H],
                       n2_w_p.rearrange("l (k p) h -> p l k h", p=P))
        w_e = [[we_all[:, i, k, :] for k in range(3)] for i in range(L)]
        w_n1 = [[wn1_all[:, i, k, :] for k in range(2)] for i in range(L)]
        w_n2 = [[wn2_all[:, i, k, :] for k in range(2)] for i in range(L)]
        becol_a = load("becol_a", [H, L],
                       edge_b_col_p.rearrange("l h o -> h (l o)"))
        b1col_a = load("b1col_a", [H, L],
                       n1_b_col_p.rearrange("l h o -> h (l o)"))
        b2col_a = load("b2col_a", [H, L],
                       n2_b_col_p.rearrange("l h o -> h (l o)"))
        be_col = [becol_a[:, i:i + 1] for i in range(L)]
        b1_col = [b1col_a[:, i:i + 1] for i in range(L)]
        b2_col = [b2col_a[:, i:i + 1] for i in range(L)]
        bng = load("bng", [H, 1], bn_node_g[:, :])
        bnb = load("bnb", [H, 1], bn_node_b[:, :])
        beg = load("beg", [H, 1], bn_edge_g[:, :])
        beb = load("beb", [H, 1], bn_edge_b[:, :])
        regw_h = load("regw_h", [P, 1], reg_w_p[0:P, :])
        regw_e = load("regw_e", [P, 1], reg_w_p[P:2 * P, :])
        regb_sb = load("regb_sb", [1, 1], reg_b_p[:, :])

        # bf16 copies of raw chunk-1 weights (never folded)
        n1b1, wn2b1 = [], []
        for i in range(L):
            t = singles.tile([P, H], BF16, tag=f"n1b1_{i}", name=f"n1b1_{i}")
            nc.vector.tensor_copy(out=t[:], in_=w_n1[i][1])
            n1b1.append(t)
            t2 = singles.tile([P, H], BF16, tag=f"wn2b1_{i}", name=f"wn2b1_{i}")
            nc.vector.tensor_copy(out=t2[:], in_=w_n2[i][1])
            wn2b1.append(t2)

        hT = [
            singles.tile([P, NPC], BF16, tag=f"hT_{s}", name=f"hT_{s}")
            for s in range(2)
        ]
        eT = singles.tile([P, ECAP], BF16, tag="eT", name="eT")

        def copy_dve(dst, src):
            nc.vector.tensor_copy(out=dst, in_=src)

        # ================= encoder =================
        nsl = [(s, min(s + 512, NPC)) for s in range(0, NPC, 512)]
        for (s0, s1) in nsl:
            xsl = wpool.tile([F, 512], BF16, tag="xsl", name="xsl")
            nc.sync.dma_start(out=xsl[:, :s1 - s0], in_=xT[:, s0:s1])
            pse = ps_e.tile([P, 512], F32, tag="pse_w", name="pse")
            nc.tensor.matmul(
                out=pse[:, :s1 - s0], lhsT=encn_w[:], rhs=xsl[:, :s1 - s0],
                start=True, stop=True,
            )
            nc.scalar.activation(
                out=hT[0][:, s0:s1], in_=pse[:, :s1 - s0], func=Act.Relu,
            )
        # node-major hseg windows (pads already 0: x=0, bias=0);
        # 4 windows per copy/DMA via a row-grouped view of hseg
        hsegR = [t.rearrange("(a p) h -> p a h", p=P) for t in hseg]
        for w4 in range(0, NW, 4):
            nw4 = min(4, NW - w4)
            psT = ps_t.tile([P, 512], BF16, tag="psT", name="psT")
            for j4 in range(nw4):
                ws = (w4 + j4) * P
                nc.tensor.transpose(
                    out=psT[:, j4 * P:(j4 + 1) * P], in_=hT[0][:, ws:ws + P],
                    identity=ident_bf[:],
                )
            hwin_bf = cpool.tile([P, 512], BF16, tag="hwin_bf", name="hwin_bf")
            nc.scalar.activation(
                out=hwin_bf[:, :nw4 * P], in_=psT[:, :nw4 * P], func=Act.Copy
            )
            nc.sync.dma_start(
                out=hsegR[0][:, w4:w4 + nw4, :],
                in_=hwin_bf[:, :nw4 * P],
            )
        nc.sync.dma_start(out=hseg[0][NPC:NPCT, :], in_=zero_bf[:])
        nc.gpsimd.collective_compute(
            "AllGather", AluOp.bypass, replica_groups=rg,
            ins=[hseg[0][:, :]], outs=[htab[0][:, :]],
        )
        # edge-attr encoding: overlaps the AllGather (4 windows per DMA)
        for w4 in range(0, NW, 4):
            nw4 = min(4, NW - w4)
            es4 = w4 * WSL
            ea_sb = wpool.tile([F, 4 * WSL], BF16, tag="ea_sb", name="ea_sb")
            nc.sync.dma_start(
                out=ea_sb[:, :nw4 * WSL], in_=eaT[:, es4:es4 + nw4 * WSL]
            )
            for j4 in range(nw4):
                es = es4 + j4 * WSL
                pse2 = ps_e.tile([P, 512], F32, tag="pse_w", name="pse2")
                nc.tensor.matmul(
                    out=pse2[:], lhsT=ence_w[:],
                    rhs=ea_sb[:, j4 * WSL:(j4 + 1) * WSL],
                    start=True, stop=True,
                )
                nc.scalar.activation(
                    out=eT[:, es:es + WSL], in_=pse2[:], func=Act.Relu,
                )

        # ================= layers =================
        epad_bf = spool.tile([P, 1], BF16, tag="epad_bf_a", name="epad_bf")
        nc.vector.memset(epad_bf[:], 0.0)
        hpad_bf = spool.tile([P, 1], BF16, tag="hpad_bf_a", name="hpad_bf")
        nc.vector.memset(hpad_bf[:], 0.0)
        s_h = t_h = s_e = t_e = None
        for i in range(L):
            last = i == L - 1
            h_cur, h_nxt = hT[i % 2], hT[(i + 1) % 2]

            # ---- fold BN into this layer's weights (bf16 outputs) ----
            if i == 0:
                wef0 = spool.tile([P, H], BF16, tag="wef_0", name="wef_0")
                nc.vector.tensor_copy(out=wef0[:], in_=w_e[0][0])
                wef1b = spool.tile([P, H], BF16, tag="wef_1", name="wef_1")
                nc.vector.tensor_copy(out=wef1b[:], in_=w_e[0][1])
                wef2 = spool.tile([P, H], BF16, tag="wef_2", name="wef_2")
                nc.vector.tensor_copy(out=wef2[:], in_=w_e[0][2])
                n1f0 = spool.tile([P, H], BF16, tag="n1f0", name="n1f0")
                nc.vector.tensor_copy(out=n1f0[:], in_=w_n1[0][0])
                n2f0 = spool.tile([P, H], BF16, tag="n2f0", name="n2f0")
                nc.vector.tensor_copy(out=n2f0[:], in_=w_n2[0][0])
                bef, b1f, b2f = be_col[0], b1_col[0], b2_col[0]
                b1bc = None
            else:
                def fold(dst_tag, src, scol):
                    t = spool.tile([P, H], BF16, tag=dst_tag, name=dst_tag)
                    nc.vector.tensor_scalar(
                        out=t[:], in0=src, scalar1=scol[:, 0:1],
                        scalar2=None, op0=AluOp.mult,
                    )
                    return t

                wef0 = fold("wef_0", w_e[i][0], s_h)
                wef1b = fold("wef_1", w_e[i][1], s_h)
                wef2 = fold("wef_2", w_e[i][2], s_e)
                n1f0 = fold("n1f0", w_n1[i][0], s_h)
                n2f0 = fold("n2f0", w_n2[i][0], s_h)
                psb = ps_x.tile([P, P], F32, tag="mix", name="psb")
                nc.tensor.matmul(out=psb[:, 0:1], lhsT=w_e[i][0],
                                 rhs=t_h[:, 0:1], start=True, stop=False)
                nc.tensor.matmul(out=psb[:, 0:1], lhsT=w_e[i][1],
                                 rhs=t_h[:, 0:1], start=False, stop=False)
                nc.tensor.matmul(out=psb[:, 0:1], lhsT=w_e[i][2],
                                 rhs=t_e[:, 0:1], start=False, stop=True)
                bef = spool.tile([H, 1], F32, tag="bef", name="bef")
                nc.vector.tensor_tensor(
                    out=bef[:], in0=psb[:, 0:1], in1=be_col[i], op=AluOp.add
                )
                psb1 = ps_x.tile([P, P], F32, tag="mix", name="psb1")
                nc.tensor.matmul(out=psb1[:, 0:1], lhsT=w_n1[i][0],
                                 rhs=t_h[:, 0:1], start=True, stop=True)
                b1f = spool.tile([H, 1], F32, tag="b1f", name="b1f")
                nc.vector.tensor_tensor(
                    out=b1f[:], in0=psb1[:, 0:1], in1=b1_col[i], op=AluOp.add
                )
                psb2 = ps_x.tile([P, P], F32, tag="mix", name="psb2")
                nc.tensor.matmul(out=psb2[:, 0:1], lhsT=w_n2[i][0],
                                 rhs=t_h[:, 0:1], start=True, stop=True)
                b2f = spool.tile([H, 1], F32, tag="b2f", name="b2f")
                nc.vector.tensor_tensor(
                    out=b2f[:], in0=psb2[:, 0:1], in1=b2_col[i], op=AluOp.add
                )

                # broadcast b1f to [P, 512] for the edge-major psm add
                psr = ps_x.tile([P, P], F32, tag="mix", name="psr")
                nc.tensor.transpose(
                    out=psr[0:1, 0:P], in_=b1f[:, 0:1], identity=ident_f[:]
                )
                rowt4 = spool.tile([1, 512], F32, tag="b1_row", name="b1_row")
                for q in range(4):
                    copy_dve(rowt4[:, q * P:(q + 1) * P], psr[0:1, 0:P])
                psb_ = ps_m.tile([P, 512], F32, tag="psm", name="psb_")
                nc.tensor.matmul(
                    out=psb_[:], lhsT=ones_row[:], rhs=rowt4[:],
                    start=True, stop=True,
                )
                b1bc = spool.tile([P, 512], F32, tag="b1_bc", name="b1_bc")
                copy_dve(b1bc[:], psb_[:])

            # pad-slot e value for this layer (mirrors the stored bf16 chain)
            pspad = ps_x.tile([P, P], F32, tag="mix", name="pspad")
            nc.tensor.matmul(
                out=pspad[:, 0:1], lhsT=wef2[:], rhs=epad_bf[:, 0:1],
                start=True, stop=True,
            )
            epad_f = spool.tile([P, 1], F32, tag="epad_f", name="epad_f")
            nc.scalar.activation(
                out=epad_f[:], in_=pspad[:, 0:1], func=Act.Relu, bias=bef[:, 0:1]
            )
            epad_bf = spool.tile(
                [P, 1], BF16, tag=f"epad_bf_{'b' if i % 2 == 0 else 'a'}",
                name="epad_bf2",
            )
            nc.vector.tensor_copy(out=epad_bf[:], in_=epad_f[:])
            epx = spool.tile([P, 1], F32, tag="epx", name="epx")
            nc.vector.tensor_copy(out=epx[:], in_=epad_bf[:])
            # pad-node h value chain (agg contribution is 0 for pad nodes)
            pspadh = ps_x.tile([P, P], F32, tag="mix", name="pspadh")
            nc.tensor.matmul(
                out=pspadh[:, 0:1], lhsT=n2f0[:], rhs=hpad_bf[:, 0:1],
                start=True, stop=True,
            )
            hpad_f = spool.tile([P, 1], F32, tag="hpad_f", name="hpad_f")
            nc.scalar.activation(
                out=hpad_f[:], in_=pspadh[:, 0:1], func=Act.Relu, bias=b2f[:, 0:1]
            )
            hpad_bf = spool.tile(
                [P, 1], BF16, tag=f"hpad_bf_{'b' if i % 2 == 0 else 'a'}",
                name="hpad_bf2",
            )
            nc.vector.tensor_copy(out=hpad_bf[:], in_=hpad_f[:])
            hpx = spool.tile([P, 1], F32, tag="hpx", name="hpx")
            nc.vector.tensor_copy(out=hpx[:], in_=hpad_bf[:])

            # ---- stats accumulators ----
            se_cols = spool.tile([P, NW], F32, tag="se_cols", name="se_cols")
            se2_cols = spool.tile([P, NW], F32, tag="se2_cols", name="se2_cols")
            sh_cols = spool.tile([P, NG], F32, tag="sh_cols", name="sh_cols")
            sh2_cols = spool.tile([P, NG], F32, tag="sh2_cols", name="sh2_cols")

            # ---- window sweep ----
            off_lo = off_hi = 0
            for g in range(NG):
                w0 = g * GW
                gsz = min(GW, NW - w0)
                nlo, nhi = gsz * LOSL, gsz * HISL
                glo = gpool.tile([P, gsz * CLO, P], BF16, tag="glo", name="glo")
                ghi = gpool.tile([P, gsz * (CPW - CLO), P], BF16, tag="ghi",
                                 name="ghi")
                with tc.high_priority():
                    nc.gpsimd.dma_gather(
                        out_ap=glo[:], in_ap=htab[i][0:LO_LIM, :],
                        idxs_ap=ipr_lo_sb[:, off_lo:off_lo + nlo // 16],
                        num_idxs=nlo, num_idxs_reg=nlo, elem_size=H,
                        transpose=False, single_packet=False,
                        queue_num=(2 * g) % 4,
                    )
                    nc.gpsimd.dma_gather(
                        out_ap=ghi[:], in_ap=htab[i][HI_BASE:NPADT, :],
                        idxs_ap=ipr_hi_sb[:, off_hi:off_hi + nhi // 16],
                        num_idxs=nhi, num_idxs_reg=nhi, elem_size=H,
                        transpose=False, single_packet=False,
                        queue_num=(2 * g + 1) % 4,
                    )
                off_lo += nlo // 16
                off_hi += nhi // 16
                aggT_big = cpool.tile([P, 512], BF16, tag="aggT",
                                      name="aggT_big")
                for j in range(gsz):
                    w = w0 + j
                    ws, es = w * P, w * WSL
                    # feature-major h[row] for this window
                    hrT = wpool.tile([P, WSL], BF16, tag="hrT", name="hrT")
                    with tc.high_priority():
                        psT = ps_t.tile([P, 512], BF16, tag="psT", name="psT")
                        for ck in range(CPW):
                            src = (glo[:, j * CLO + ck, :] if ck < CLO
                                   else ghi[:, j * (CPW - CLO) + (ck - CLO), :])
                            nc.tensor.transpose(
                                out=psT[:, ck * P:(ck + 1) * P], in_=src,
                                identity=ident_bf[:],
                            )
                        copy_dve(hrT[:], psT[:])

                    ohne_w = wpool.tile([P, WSL], BF16, tag="ohne_w",
                                        name="ohne_w")
                    nc.sync.dma_start(out=ohne_w[:], in_=ohne_p[:, es:es + WSL])

                    # zT = (h_win)^T @ We1_folded   [node, H]
                    with tc.high_priority():
                        psz = ps_x.tile([P, P], F32, tag="mix", name="psz")
                        nc.tensor.matmul(
                            out=psz[:], lhsT=h_cur[:, ws:ws + P],
                            rhs=wef1b[:], start=True, stop=True,
                        )
                        zT = cpool.tile([P, P], BF16, tag="zT", name="zT")
                        copy_dve(zT[:], psz[:])

                    pse = ps_e.tile([P, 512], F32, tag="pse_w", name="pse")
                    nc.tensor.matmul(
                        out=pse[:], lhsT=wef0[:], rhs=hrT[:],
                        start=True, stop=False,
                    )
                    nc.tensor.matmul(
                        out=pse[:], lhsT=wef2[:], rhs=eT[:, es:es + WSL],
                        start=False, stop=False,
                    )
                    nc.tensor.matmul(
                        out=pse[:], lhsT=zT[:], rhs=ohne_w[:],
                        start=False, stop=True,
                    )
                    # in-place e update (WAR on the pse read is tracked)
                    nc.scalar.activation(
                        out=eT[:, es:es + WSL], in_=pse[:], func=Act.Relu,
                        bias=bef[:, 0:1], accum_out=se_cols[:, w:w + 1],
                    )
                    if not last:
                        dump_e = wpool.tile([P, WSL], BF16, tag="dump_e",
                                            name="dump_e")
                        nc.scalar.activation(
                            out=dump_e[:], in_=eT[:, es:es + WSL],
                            func=Act.Square, accum_out=se2_cols[:, w:w + 1],
                        )

                    psa = ps_a.tile([P, P], F32, tag="psa", name="psa")
                    psmb = ps_m.tile([P, 512], F32, tag="psm", name="psm")
                    for ck in range(CPW):
                        cs = es + ck * P
                        kg = w * CPW + ck
                        psm = psmb[:, ck * P:(ck + 1) * P]
                        nc.tensor.matmul(
                            out=psm, lhsT=hrT[:, ck * P:(ck + 1) * P],
                            rhs=n1f0[:], start=True, stop=False,
                        )
                        nc.tensor.matmul(
                            out=psm, lhsT=eT[:, cs:cs + P],
                            rhs=n1b1[i][:], start=False, stop=True,
                        )
                    if b1bc is not None:
                        nc.vector.tensor_tensor(
                            out=psmb[:], in0=psmb[:], in1=b1bc[:],
                            op=AluOp.add,
                        )
                    m_big = cpool.tile([P, 512], BF16, tag="m_sb",
                                       name="m_sb")
                    nc.scalar.activation(out=m_big[:], in_=psmb[:],
                                         func=Act.Relu)
                    for ck in range(CPW):
                        kg = w * CPW + ck
                        nc.tensor.matmul(
                            out=psa[:], lhsT=m_big[:, ck * P:(ck + 1) * P],
                            rhs=onehot_sb[:, kg * P:(kg + 1) * P],
                            start=(ck == 0), stop=(ck == CPW - 1),
                        )
                    nc.scalar.activation(
                        out=aggT_big[:, j * P:(j + 1) * P], in_=psa[:],
                        func=Act.Copy,
                    )

                # ---- group tail: n2 over gsz windows in two wide matmuls ----
                gws, gwid = w0 * P, gsz * P
                psf = ps_e.tile([P, 512], F32, tag="pse_w", name="psf")
                nc.tensor.matmul(
                    out=psf[:, :gwid], lhsT=n2f0[:],
                    rhs=h_cur[:, gws:gws + gwid], start=True, stop=False,
                )
                nc.tensor.matmul(
                    out=psf[:, :gwid], lhsT=wn2b1[i][:],
                    rhs=aggT_big[:, :gwid], start=False, stop=True,
                )
                nc.scalar.activation(
                    out=h_nxt[:, gws:gws + gwid], in_=psf[:, :gwid],
                    func=Act.Relu, bias=b2f[:, 0:1],
                    accum_out=sh_cols[:, g:g + 1],
                )
                if not last:
                    dump_h = cpool.tile([P, 512], BF16, tag="dump_h",
                                        name="dump_h")
                    nc.scalar.activation(
                        out=dump_h[:, :gwid], in_=h_nxt[:, gws:gws + gwid],
                        func=Act.Square, accum_out=sh2_cols[:, g:g + 1],
                    )
                    # node-major windows for the table; pad-node rows are
                    # never gathered, so no masking is needed
                    psh = ps_t.tile([P, 512], BF16, tag="psT", name="psh")
                    for j in range(gsz):
                        ws = (w0 + j) * P
                        nc.tensor.transpose(
                            out=psh[:, j * P:(j + 1) * P],
                            in_=h_nxt[:, ws:ws + P], identity=ident_bf[:],
                        )
                    hwin_bf = cpool.tile([P, 512], BF16, tag="hwin_bf",
                                         name="hwin_bf")
                    nc.scalar.activation(
                        out=hwin_bf[:, :gwid], in_=psh[:, :gwid], func=Act.Copy
                    )
                    nc.sync.dma_start(
                        out=hsegR[i + 1][:, w0:w0 + gsz, :],
                        in_=hwin_bf[:, :gwid],
                    )

            # ---- end of layer: AllGather next table, then stats ----
            if not last:
                nc.sync.dma_start(out=hseg[i + 1][NPC:NPCT, :], in_=zero_bf[:])
                nc.gpsimd.collective_compute(
                    "AllGather", AluOp.bypass, replica_groups=rg,
                    ins=[hseg[i + 1][:, :]], outs=[htab[i + 1][:, :]],
                )

            # ---- stats AllReduce ----
            ar_sb = spool.tile([P, 4], F32, tag="ar_sb", name="ar_sb")

            def stat_col(dst, cols, pad_val, pad_cnt):
                nc.vector.tensor_reduce(
                    out=dst, in_=cols[:], axis=mybir.AxisListType.X,
                    op=AluOp.add,
                )
                cor = spool.tile([P, 1], F32, tag="cor", name="cor")
                nc.vector.tensor_tensor(
                    out=cor[:], in0=pad_val, in1=pad_cnt, op=AluOp.mult
                )
                nc.vector.tensor_tensor(
                    out=dst, in0=dst, in1=cor[:], op=AluOp.subtract
                )

            # sums read the stored bf16 values -> corrections use the
            # bf16-rounded pad values (epx/hpx)
            stat_col(ar_sb[:, 0:1], se_cols, epx[:], npads_sb[:])
            stat_col(ar_sb[:, 2:3], sh_cols, hpx[:], npadn_sb[:])
            if not last:
                esq = spool.tile([P, 1], F32, tag="esq", name="esq")
                nc.vector.tensor_tensor(
                    out=esq[:], in0=epx[:], in1=epx[:], op=AluOp.mult
                )
                stat_col(ar_sb[:, 1:2], se2_cols, esq[:], npads_sb[:])
                hsq = spool.tile([P, 1], F32, tag="hsq", name="hsq")
                nc.vector.tensor_tensor(
                    out=hsq[:], in0=hpx[:], in1=hpx[:], op=AluOp.mult
                )
                stat_col(ar_sb[:, 3:4], sh2_cols, hsq[:], npadn_sb[:])
            else:
                nc.vector.memset(ar_sb[:, 1:2], 0.0)
                nc.vector.memset(ar_sb[:, 3:4], 0.0)
            nc.sync.dma_start(out=ar_in[i][:, :], in_=ar_sb[:])
            nc.gpsimd.collective_compute(
                "AllReduce", AluOp.add, replica_groups=rg,
                ins=[ar_in[i][:, :]], outs=[ar_out[i][:, :]],
            )
            arr = spool.tile([P, 4], F32, tag="arr", name="arr")
            nc.sync.dma_start(out=arr[:], in_=ar_out[i][:, :])

            if not last:
                def bn_consts(sum_c, sq_c, inv_cnt, g_t, b_t, tag):
                    mean = spool.tile([P, 1], F32, tag=f"mean_{tag}",
                                      name=f"mean_{tag}")
                    nc.vector.tensor_scalar(
                        out=mean[:], in0=sum_c, scalar1=inv_cnt, scalar2=None,
                        op0=AluOp.mult,
                    )
                    var = spool.tile([P, 1], F32, tag=f"var_{tag}",
                                     name=f"var_{tag}")
                    nc.vector.tensor_scalar(
                        out=var[:], in0=sq_c, scalar1=inv_cnt, scalar2=None,
                        op0=AluOp.mult,
                    )
                    m2 = spool.tile([P, 1], F32, tag=f"m2_{tag}",
                                    name=f"m2_{tag}")
                    nc.vector.tensor_tensor(
                        out=m2[:], in0=mean[:], in1=mean[:], op=AluOp.mult
                    )
                    nc.vector.tensor_tensor(
                        out=var[:], in0=var[:], in1=m2[:], op=AluOp.subtract
                    )
                    sd = spool.tile([P, 1], F32, tag=f"sd_{tag}",
                                    name=f"sd_{tag}")
                    nc.scalar.activation(
                        out=sd[:], in_=var[:], func=Act.Sqrt,
                        bias=eps_sb[:, 0:1],
                    )
                    rs = spool.tile([P, 1], F32, tag=f"rs_{tag}",
                                    name=f"rs_{tag}")
                    nc.vector.reciprocal(out=rs[:], in_=sd[:])
                    s = spool.tile([P, 1], F32, tag=f"s_{tag}",
                                   name=f"s_{tag}")
                    nc.vector.tensor_tensor(
                        out=s[:], in0=rs[:], in1=g_t[:], op=AluOp.mult
                    )
                    ms = spool.tile([P, 1], F32, tag=f"ms_{tag}",
                                    name=f"ms_{tag}")
                    nc.vector.tensor_tensor(
                        out=ms[:], in0=mean[:], in1=s[:], op=AluOp.mult
                    )
                    t = spool.tile([P, 1], F32, tag=f"t_{tag}",
                                   name=f"t_{tag}")
                    nc.vector.tensor_tensor(
                        out=t[:], in0=b_t[:], in1=ms[:], op=AluOp.subtract
                    )
                    return s, t

                s_e, t_e = bn_consts(arr[:, 0:1], arr[:, 1:2], inv_e, beg,
                                     beb, "e")
                s_h, t_h = bn_consts(arr[:, 2:3], arr[:, 3:4], inv_n, bng,
                                     bnb, "h")
            else:
                roh = spool.tile([P, 1], F32, tag="roh", name="roh")
                nc.vector.tensor_scalar(
                    out=roh[:], in0=arr[:, 2:3], scalar1=inv_n, scalar2=None,
                    op0=AluOp.mult,
                )
                roe = spool.tile([P, 1], F32, tag="roe", name="roe")
                nc.vector.tensor_scalar(
                    out=roe[:], in0=arr[:, 0:1], scalar1=inv_e, scalar2=None,
                    op0=AluOp.mult,
                )
                pso = ps_x.tile([P, P], F32, tag="mix", name="pso")
                nc.tensor.matmul(
                    out=pso[0:1, 0:1], lhsT=roh[:, 0:1], rhs=regw_h[:, 0:1],
                    start=True, stop=False,
                )
                nc.tensor.matmul(
                    out=pso[0:1, 0:1], lhsT=roe[:, 0:1], rhs=regw_e[:, 0:1],
                    start=False, stop=True,
                )
                out_sb = spool.tile([1, 1], F32, tag="out_sb", name="out_sb")
                nc.vector.tensor_tensor(
                    out=out_sb[:], in0=pso[0:1, 0:1], in1=regb_sb[:],
                    op=AluOp.add,
                )
                nc.sync.dma_start(out=out_p[:, :], in_=out_sb[:])

    nc.compile()
    return nc


def kernel(**inputs) -> np.ndarray:
    cfg = Cfg()
    maps = prep(cfg, inputs["x"], inputs["edge_index"], inputs["edge_attr"])
    add_weight_params(cfg, maps, inputs)
    nc = build(cfg)
    from concourse.bass_utils import run_bass_kernel_spmd

    res = run_bass_kernel_spmd(nc, maps, list(range(cfg.NC)))
    return np.asarray(res.results[0]["out"], np.float32)


# revision 43
# speedup vs baseline: 1.0059x; 1.0059x over previous
"""Trainium2 Bass kernel for nn_BaseMPNN (GNN message passing), 8-core SPMD.

Design (vs the v2 pair-packed baseline):
- Gather: two plain-mode dma_gathers per 4-window group over single 256B
  node rows (lo: htab[0:32768] idx=row; hi: htab[18432:] idx=row-18432).
  Edges whose source row falls in the overlap [18432,32768) go to whichever
  section balances both to exactly 2 chunks of 128. No pair packing, no
  parity-select DVE chain, half the gather bytes.
- Nodes are LPT-balanced across cores/windows (assignment is pure data:
  onehot / index metadata), so every window holds <=512 edges in 4 chunks
  (2 lo + 2 hi) instead of 5 -> ~20% less edge work, ~0.35% slot padding.
- onehot (scatter rhs, carries rdeg so no separate scale) and eT (edge
  features, updated in place) are SBUF-resident all 3 layers; ohne streams.
- h table, h state, and all matmuls are bf16 (folded BN weights cast).
- Stats accumulate during the sweep via activation accum_out plus pad
  correction chains (epad for e, hpad for pad nodes), so no end-of-layer
  stats matmul tail.
"""

import math
from contextlib import ExitStack
from dataclasses import dataclass

import numpy as np
import ml_dtypes

import concourse.bacc as bacc
import concourse.bass as bass
import concourse.tile as tile
from concourse import mybir
from concourse.masks import make_identity

F32 = mybir.dt.float32
BF16 = mybir.dt.bfloat16
I16 = mybir.dt.int16
P = 128
EPS = 1e-5
BF = ml_dtypes.bfloat16

LO_LIM = 32768        # lo gather covers table rows [0, 32768)
HI_BASE = 18432       # hi gather covers table rows [18432, NPADT)


@dataclass(frozen=True)
class Cfg:
    NC: int = 8        # cores
    H: int = 128       # hidden (must be 128)
    F: int = 16        # input features
    L: int = 3         # meta layers
    NW: int = 49       # 128-node windows per core
    CLO: int = 2       # lo chunks per window
    CHI: int = 2       # hi chunks per window
    GW: int = 4        # windows per gather group
    N_real: int = 50000
    E_real: int = 200000

    @property
    def CPW(self):
        return self.CLO + self.CHI

    @property
    def NPC(self):   # compute nodes per core
        return self.NW * P

    @property
    def NPCT(self):  # node-table rows per core (incl. zero pad window)
        return (self.NW + 1) * P

    @property
    def NPADT(self):  # global node-table rows
        return self.NC * self.NPCT

    @property
    def ECAP(self):  # edge slots per core
        return self.NW * self.CPW * P

    @property
    def CHUNKS(self):
        return self.NW * self.CPW

    @property
    def NG(self):    # gather groups
        return math.ceil(self.NW / self.GW)

    @property
    def ZLO(self):   # guaranteed-zero table row for lo-section pads
        return self.NPC

    @property
    def ZHI(self):   # guaranteed-zero table row for hi-section pads
        return (self.NC - 1) * self.NPCT + self.NPC


def _wrap16(flat):
    """int16 flat index list -> [128, n/16] wrap-16, replicated x8 groups."""
    n = len(flat)
    assert n % 16 == 0
    w = flat.reshape(n // 16, 16).T
    return np.ascontiguousarray(np.tile(w, (8, 1)))


def _balance_nodes(cfg: Cfg, col):
    """LPT-pack nodes into NC*NW windows (<=128 nodes, ~equal edge counts).

    Returns (node_wg, node_slot): window-global id and slot for each node.
    """
    import heapq

    NBINS = cfg.NC * cfg.NW
    deg = np.bincount(col, minlength=cfg.N_real)
    order = np.argsort(-deg, kind="stable")
    node_wg = np.empty(cfg.N_real, np.int64)
    node_slot = np.empty(cfg.N_real, np.int64)
    heap = [(0, b) for b in range(NBINS)]
    heapq.heapify(heap)
    nodes_in = np.zeros(NBINS, np.int64)
    for n in order:
        edges, b = heapq.heappop(heap)
        node_wg[n] = b
        node_slot[n] = nodes_in[b]
        nodes_in[b] += 1
        if nodes_in[b] < P:
            heapq.heappush(heap, (edges + int(deg[n]), b))
    return node_wg, node_slot


def prep(cfg: Cfg, x, edge_index, edge_attr):
    """Host-side preprocessing -> per-core input maps."""
    x = np.asarray(x, np.float32)
    ei = np.asarray(edge_index, np.int64)
    ea = np.asarray(edge_attr, np.float32)
    row, col = ei[0], ei[1]
    NPC, NW, CPW, CLO, ECAP, CHUNKS = (
        cfg.NPC, cfg.NW, cfg.CPW, cfg.CLO, cfg.ECAP, cfg.CHUNKS,
    )
    LOSL, HISL = cfg.CLO * P, cfg.CHI * P

    deg = np.bincount(col, minlength=cfg.N_real).astype(np.float32)
    rdeg_all = 1.0 / np.maximum(deg, 1.0)

    node_wg, node_slot = _balance_nodes(cfg, col)
    node_core = node_wg // NW
    node_wl = node_wg % NW
    tid = node_core * cfg.NPCT + node_wl * P + node_slot  # global table row

    e_wg = node_wg[col]
    e_order = np.argsort(e_wg, kind="stable")
    ewg_sorted = e_wg[e_order]
    wstart = np.searchsorted(ewg_sorted, np.arange(cfg.NC * NW))
    wend = np.searchsorted(ewg_sorted, np.arange(cfg.NC * NW) + 1)

    maps = []
    for c in range(cfg.NC):
        # per-slot metadata, linear slot index = chunk*128 + partition
        idx_lo = np.full(NW * LOSL, cfg.ZLO, np.int64)
        idx_hi = np.full(NW * HISL, cfg.ZHI - HI_BASE, np.int64)
        oh_n = np.full(ECAP, -1, np.int64)    # col node slot (-1 = pad)
        redge = np.zeros(ECAP, np.float32)
        ea_slots = np.zeros((ECAP, cfg.F), np.float32)

        nodes_c = int((node_core == np.int64(c)).sum())
        for w in range(NW):
            wg = c * NW + w
            sel = e_order[wstart[wg]:wend[wg]]
            erow_t = tid[row[sel]]
            is_lof = erow_t < HI_BASE
            is_hif = erow_t >= LO_LIM
            is_flex = ~is_lof & ~is_hif
            n_lof, n_hif = int(is_lof.sum()), int(is_hif.sum())
            n_flex = int(is_flex.sum())
            assert n_lof + n_hif + n_flex <= CPW * P, (c, w)
            assert n_lof <= LOSL and n_hif <= HISL, (c, w, n_lof, n_hif)
            lo_take = min(n_flex, LOSL - n_lof)
            assert n_hif + (n_flex - lo_take) <= HISL, (c, w)
            flex_idx = np.nonzero(is_flex)[0]
            lo_sel = np.concatenate([np.nonzero(is_lof)[0], flex_idx[:lo_take]])
            hi_sel = np.concatenate([np.nonzero(is_hif)[0], flex_idx[lo_take:]])

            for base_chunk, ssel, ibuf, ioff, rebase in (
                (0, lo_sel, idx_lo, w * LOSL, 0),
                (CLO, hi_sel, idx_hi, w * HISL, HI_BASE),
            ):
                cnt = len(ssel)
                eids = sel[ssel]
                ibuf[ioff:ioff + cnt] = erow_t[ssel] - rebase
                pos = np.arange(cnt)
                slot = (w * CPW + base_chunk + pos // P) * P + pos % P
                oh_n[slot] = node_slot[col[eids]]
                redge[slot] = rdeg_all[col[eids]]
                ea_slots[slot] = ea[eids]

        assert idx_lo.max() < LO_LIM and idx_lo.min() >= 0
        assert idx_hi.max() <= 32767 and idx_hi.min() >= 0

        # group-wrapped gather index buffers
        def wrap_groups(ibuf, secsl):
            parts = []
            for g in range(cfg.NG):
                w0 = g * cfg.GW
                gsz = min(cfg.GW, NW - w0)
                seg = ibuf[w0 * secsl:(w0 + gsz) * secsl].astype(np.int16)
                parts.append(_wrap16(seg))
            return np.concatenate(parts, axis=1)

        ipr_lo = wrap_groups(idx_lo, LOSL)
        ipr_hi = wrap_groups(idx_hi, HISL)

        # onehot [e-part, chunk*128 + node-slot] carries rdeg (t1w semantics,
        # so the scatter matmul needs no separate rdeg scale); ohne is 0/1.
        slots = np.arange(ECAP)
        p_arr, k_arr = slots % P, slots // P
        onehot = np.zeros((P, ECAP), BF)
        valid = oh_n >= 0
        onehot[p_arr[valid], k_arr[valid] * P + oh_n[valid]] = redge[valid]
        ohne = np.zeros((P, ECAP), BF)
        ohne[oh_n[valid], k_arr[valid] * P + p_arr[valid]] = 1

        csel = node_core == np.int64(c)
        xT = np.zeros((cfg.F, NPC), BF)
        xT[:, node_wl[csel] * P + node_slot[csel]] = x[csel].T
        maskw = np.zeros((P, NW), np.float32)
        maskw[node_slot[csel], node_wl[csel]] = 1.0

        n_edges = int(wend[c * NW + NW - 1] - wstart[c * NW])
        maps.append(
            {
                "npads": np.full((P, 1), float(ECAP - n_edges), np.float32),
                "npadn": np.full((P, 1), float(NPC - nodes_c), np.float32),
                "xT": np.ascontiguousarray(xT),
                "eaT": np.ascontiguousarray(ea_slots.T.astype(BF)),
                "ipr_lo": ipr_lo,
                "ipr_hi": ipr_hi,
                "onehot": np.ascontiguousarray(onehot),
                "ohne": np.ascontiguousarray(ohne),
                "maskw": maskw,
            }
        )
    return maps


def add_weight_params(cfg: Cfg, maps, w):
    """Append (replicated) weight arrays to each core's input map."""
    H, L = cfg.H, cfg.L

    def col(a):
        return np.asarray(a, np.float32).reshape(H, 1)

    shared = {
        "enc_node_w": np.asarray(w["enc_node_w"], np.float32).astype(BF),
        "enc_edge_w": np.asarray(w["enc_edge_w"], np.float32).astype(BF),
        "edge_w": np.asarray(w["edge_w"], np.float32),
        "edge_b_col": np.asarray(w["edge_b"], np.float32).reshape(L, H, 1),
        "n1_w": np.asarray(w["n1_w"], np.float32),
        "n1_b_col": np.asarray(w["n1_b"], np.float32).reshape(L, H, 1),
        "n2_w": np.asarray(w["n2_w"], np.float32),
        "n2_b_col": np.asarray(w["n2_b"], np.float32).reshape(L, H, 1),
        "bn_node_g": col(w["bn_node_g"]),
        "bn_node_b": col(w["bn_node_b"]),
        "bn_edge_g": col(w["bn_edge_g"]),
        "bn_edge_b": col(w["bn_edge_b"]),
        "reg_w": np.asarray(w["reg_w"], np.float32).reshape(2 * H, 1),
        "reg_b": np.asarray(w["reg_b"], np.float32).reshape(1, 1),
    }
    for k in ["enc_node_b", "enc_edge_b", "edge_b", "n1_b", "n2_b"]:
        assert np.all(np.asarray(w[k]) == 0.0), f"nonzero bias {k} unsupported"
    for m in maps:
        m.update(shared)
    return maps


def build(cfg: Cfg):
    """Build the SPMD Bass program. Returns nc."""
    H, F, L, NW, GW = cfg.H, cfg.F, cfg.L, cfg.NW, cfg.GW
    NPC, NPCT, NPADT, ECAP = cfg.NPC, cfg.NPCT, cfg.NPADT, cfg.ECAP
    NG, CHUNKS, CPW, CLO = cfg.NG, cfg.CHUNKS, cfg.CPW, cfg.CLO
    WSL = CPW * P       # 512 edge slots per window
    LOSL, HISL = cfg.CLO * P, cfg.CHI * P
    inv_n = 1.0 / cfg.N_real
    inv_e = 1.0 / cfg.E_real

    nc = bacc.Bacc(
        "TRN2", target_bir_lowering=False, debug=False, num_devices=cfg.NC,
        num_swdge_queues=4,
    )

    def param(name, shape, dt=F32):
        return nc.declare_dram_parameter(name, list(shape), dt, isOutput=False).ap()

    xT = param("xT", [F, NPC], BF16)
    eaT = param("eaT", [F, ECAP], BF16)
    ipr_lo_p = param("ipr_lo", [P, NW * LOSL // 16], I16)
    ipr_hi_p = param("ipr_hi", [P, NW * HISL // 16], I16)
    onehot_p = param("onehot", [P, ECAP], BF16)
    ohne_p = param("ohne", [P, ECAP], BF16)
    maskw_p = param("maskw", [P, NW])
    npads_p = param("npads", [P, 1])
    npadn_p = param("npadn", [P, 1])
    enc_node_w = param("enc_node_w", [F, H], BF16)
    enc_edge_w = param("enc_edge_w", [F, H], BF16)
    edge_w_p = param("edge_w", [L, 3 * H, H])
    edge_b_col_p = param("edge_b_col", [L, H, 1])
    n1_w_p = param("n1_w", [L, 2 * H, H])
    n1_b_col_p = param("n1_b_col", [L, H, 1])
    n2_w_p = param("n2_w", [L, 2 * H, H])
    n2_b_col_p = param("n2_b_col", [L, H, 1])
    bn_node_g = param("bn_node_g", [H, 1])
    bn_node_b = param("bn_node_b", [H, 1])
    bn_edge_g = param("bn_edge_g", [H, 1])
    bn_edge_b = param("bn_edge_b", [H, 1])
    reg_w_p = param("reg_w", [2 * H, 1])
    reg_b_p = param("reg_b", [1, 1])
    out_p = nc.declare_dram_parameter("out", [1, 1], F32, isOutput=True).ap()

    hseg = [nc.dram_tensor(f"hseg_{i}", [NPCT, H], BF16).ap() for i in range(L)]
    htab = [
        nc.dram_tensor(f"htab_{i}", [NPADT, H], BF16, addr_space="Shared").ap()
        for i in range(L)
    ]
    ar_in = [nc.dram_tensor(f"ar_in_{i}", [H, 4], F32).ap() for i in range(L)]
    ar_out = [
        nc.dram_tensor(f"ar_out_{i}", [H, 4], F32, addr_space="Shared").ap()
        for i in range(L)
    ]
    rg = [list(range(cfg.NC))]

    AluOp = mybir.AluOpType
    Act = mybir.ActivationFunctionType

    with tile.TileContext(nc) as tc, ExitStack() as ctx:
        singles = ctx.enter_context(tc.tile_pool(name="singles", bufs=1))
        spool = ctx.enter_context(tc.tile_pool(name="spool", bufs=1))
        wpool = ctx.enter_context(tc.tile_pool(name="wpool", bufs=3))
        cpool = ctx.enter_context(tc.tile_pool(name="cpool", bufs=4))
        gpool = ctx.enter_context(tc.tile_pool(name="gpool", bufs=3))
        # PSUM budget: 8 banks of [128, 2KB]. pse 2 + psT 2 + psm 2 + psa 1
        # + shared psf/mix 1.
        ps_e = ctx.enter_context(tc.tile_pool(name="ps_e", bufs=2, space="PSUM"))
        ps_t = ctx.enter_context(tc.tile_pool(name="ps_t", bufs=2, space="PSUM"))
        ps_m = ctx.enter_context(tc.tile_pool(name="ps_m", bufs=2, space="PSUM"))
        ps_a = ctx.enter_context(tc.tile_pool(name="ps_a", bufs=1, space="PSUM"))
        ps_f = ctx.enter_context(tc.tile_pool(name="ps_f", bufs=1, space="PSUM"))
        ps_x = ps_f

        ones_row = singles.tile([1, P], F32)
        nc.vector.memset(ones_row[:], 1.0)
        ident1 = singles.tile([1, 1], F32)
        nc.vector.memset(ident1[:], 1.0)
        ident_f = singles.tile([P, P], F32)
        make_identity(nc, ident_f[:])
        ident_bf = singles.tile([P, P], BF16)
        nc.vector.tensor_copy(out=ident_bf[:], in_=ident_f[:])
        eps_sb = singles.tile([P, 1], F32)
        nc.vector.memset(eps_sb[:], EPS)
        zero_bf = singles.tile([P, P], BF16)
        nc.vector.memset(zero_bf[:], 0.0)

        def load(name_, shape, src, dt=F32, pool=singles):
            t = pool.tile(shape, dt, tag=name_, name=name_)
            nc.sync.dma_start(out=t[:], in_=src)
            return t

        ipr_lo_sb = load("ipr_lo_sb", [P, NW * LOSL // 16], ipr_lo_p[:, :], I16)
        ipr_hi_sb = load("ipr_hi_sb", [P, NW * HISL // 16], ipr_hi_p[:, :], I16)
        # biggest param load (6.4MB) goes on the Scalar engine's DMA queue
        # so it doesn't serialize the sync queue ahead of the encoder loads
        onehot_sb = singles.tile([P, ECAP], BF16, tag="onehot_sb",
                                 name="onehot_sb")
        nc.scalar.dma_start(out=onehot_sb[:], in_=onehot_p[:, :])
        maskw_sb = load("maskw_sb", [P, NW], maskw_p[:, :])
        npads_sb = load("npads_sb", [P, 1], npads_p[:, :])
        npadn_sb = load("npadn_sb", [P, 1], npadn_p[:, :])
        encn_w = load("encn_w", [F, H], enc_node_w[:, :], BF16)
        ence_w = load("ence_w", [F, H], enc_edge_w[:, :], BF16)
        # one DMA per weight family (vs 21 small ones) via rearranged views
        we_all = load("we_all", [P, L, 3, H],
                      edge_w_p.rearrange("l (k p) h -> p l k h", p=P))
        wn1_all = load("wn1_all", [P, L, 2, H],
                       n1_w_p.rearrange("l (k p) h -> p l k h", p=P))
        wn2_all = load("wn2_all", [P, L, 2, H],
                       n2_w_p.rearrange("l (k p) h -> p l k h", p=P))
        w_e = [[we_all[:, i, k, :] for k in range(3)] for i in range(L)]
        w_n1 = [[wn1_all[:, i, k, :] for k in range(2)] for i in range(L)]
        w_n2 = [[wn2_all[:, i, k, :] for k in range(2)] for i in range(L)]
        becol_a = load("becol_a", [H, L],
                       edge_b_col_p.rearrange("l h o -> h (l o)"))
        b1col_a = load("b1col_a", [H, L],
                       n1_b_col_p.rearrange("l h o -> h (l o)"))
        b2col_a = load("b2col_a", [H, L],
                       n2_b_col_p.rearrange("l h o -> h (l o)"))
        be_col = [becol_a[:, i:i + 1] for i in range(L)]
        b1_col = [b1col_a[:, i:i + 1] for i in range(L)]
        b2_col = [b2col_a[:, i:i + 1] for i in range(L)]
        bng = load("bng", [H, 1], bn_node_g[:, :])
        bnb = load("bnb", [H, 1], bn_node_b[:, :])
        beg = load("beg", [H, 1], bn_edge_g[:, :])
        beb = load("beb", [H, 1], bn_edge_b[:, :])
        regw_h = load("regw_h", [P, 1], reg_w_p[0:P, :])
        regw_e = load("regw_e", [P, 1], reg_w_p[P:2 * P, :])
        regb_sb = load("regb_sb", [1, 1], reg_b_p[:, :])

        # bf16 copies of raw chunk-1 weights (never folded)
        n1b1, wn2b1 = [], []
        for i in range(L):
            t = singles.tile([P, H], BF16, tag=f"n1b1_{i}", name=f"n1b1_{i}")
            nc.vector.tensor_copy(out=t[:], in_=w_n1[i][1])
            n1b1.append(t)
            t2 = singles.tile([P, H], BF16, tag=f"wn2b1_{i}", name=f"wn2b1_{i}")
            nc.vector.tensor_copy(out=t2[:], in_=w_n2[i][1])
            wn2b1.append(t2)

        hT = [
            singles.tile([P, NPC], BF16, tag=f"hT_{s}", name=f"hT_{s}")
            for s in range(2)
        ]
        eT = singles.tile([P, ECAP], BF16, tag="eT", name="eT")

        def copy_dve(dst, src):
            nc.vector.tensor_copy(out=dst, in_=src)

        # ================= encoder =================
        nsl = [(s, min(s + 512, NPC)) for s in range(0, NPC, 512)]
        for (s0, s1) in nsl:
            xsl = wpool.tile([F, 512], BF16, tag="xsl", name="xsl")
            nc.sync.dma_start(out=xsl[:, :s1 - s0], in_=xT[:, s0:s1])
            pse = ps_e.tile([P, 512], F32, tag="pse_w", name="pse")
            nc.tensor.matmul(
                out=pse[:, :s1 - s0], lhsT=encn_w[:], rhs=xsl[:, :s1 - s0],
                start=True, stop=True,
            )
            nc.scalar.activation(
                out=hT[0][:, s0:s1], in_=pse[:, :s1 - s0], func=Act.Relu,
            )
        # node-major hseg windows (pads already 0: x=0, bias=0);
        # 4 windows per copy/DMA via a row-grouped view of hseg
        hsegR = [t.rearrange("(a p) h -> p a h", p=P) for t in hseg]
        for w4 in range(0, NW, 4):
            nw4 = min(4, NW - w4)
            psT = ps_t.tile([P, 512], BF16, tag="psT", name="psT")
            for j4 in range(nw4):
                ws = (w4 + j4) * P
                nc.tensor.transpose(
                    out=psT[:, j4 * P:(j4 + 1) * P], in_=hT[0][:, ws:ws + P],
                    identity=ident_bf[:],
                )
            hwin_bf = cpool.tile([P, 512], BF16, tag="hwin_bf", name="hwin_bf")
            nc.scalar.activation(
                out=hwin_bf[:, :nw4 * P], in_=psT[:, :nw4 * P], func=Act.Copy
            )
            nc.sync.dma_start(
                out=hsegR[0][:, w4:w4 + nw4, :],
                in_=hwin_bf[:, :nw4 * P],
            )
        nc.sync.dma_start(out=hseg[0][NPC:NPCT, :], in_=zero_bf[:])
        nc.gpsimd.collective_compute(
            "AllGather", AluOp.bypass, replica_groups=rg,
            ins=[hseg[0][:, :]], outs=[htab[0][:, :]],
        )
        # edge-attr encoding: overlaps the AllGather (4 windows per DMA)
        for w4 in range(0, NW, 4):
            nw4 = min(4, NW - w4)
            es4 = w4 * WSL
            ea_sb = wpool.tile([F, 4 * WSL], BF16, tag="ea_sb", name="ea_sb")
            nc.sync.dma_start(
                out=ea_sb[:, :nw4 * WSL], in_=eaT[:, es4:es4 + nw4 * WSL]
            )
            for j4 in range(nw4):
                es = es4 + j4 * WSL
                pse2 = ps_e.tile([P, 512], F32, tag="pse_w", name="pse2")
                nc.tensor.matmul(
                    out=pse2[:], lhsT=ence_w[:],
                    rhs=ea_sb[:, j4 * WSL:(j4 + 1) * WSL],
                    start=True, stop=True,
                )
                nc.scalar.activation(
                    out=eT[:, es:es + WSL], in_=pse2[:], func=Act.Relu,
                )

        # ================= layers =================
        epad_bf = spool.tile([P, 1], BF16, tag="epad_bf_a", name="epad_bf")
        nc.vector.memset(epad_bf[:], 0.0)
        hpad_bf = spool.tile([P, 1], BF16, tag="hpad_bf_a", name="hpad_bf")
        nc.vector.memset(hpad_bf[:], 0.0)
        s_h = t_h = s_e = t_e = None
        for i in range(L):
            last = i == L - 1
            h_cur, h_nxt = hT[i % 2], hT[(i + 1) % 2]

            # ---- fold BN into this layer's weights (bf16 outputs) ----
            if i == 0:
                wef0 = spool.tile([P, H], BF16, tag="wef_0", name="wef_0")
                nc.vector.tensor_copy(out=wef0[:], in_=w_e[0][0])
                wef1b = spool.tile([P, H], BF16, tag="wef_1", name="wef_1")
                nc.vector.tensor_copy(out=wef1b[:], in_=w_e[0][1])
                wef2 = spool.tile([P, H], BF16, tag="wef_2", name="wef_2")
                nc.vector.tensor_copy(out=wef2[:], in_=w_e[0][2])
                n1f0 = spool.tile([P, H], BF16, tag="n1f0", name="n1f0")
                nc.vector.tensor_copy(out=n1f0[:], in_=w_n1[0][0])
                n2f0 = spool.tile([P, H], BF16, tag="n2f0", name="n2f0")
                nc.vector.tensor_copy(out=n2f0[:], in_=w_n2[0][0])
                bef, b1f, b2f = be_col[0], b1_col[0], b2_col[0]
                b1bc = None
            else:
                def fold(dst_tag, src, scol):
                    t = spool.tile([P, H], BF16, tag=dst_tag, name=dst_tag)
                    nc.vector.tensor_scalar(
                        out=t[:], in0=src, scalar1=scol[:, 0:1],
                        scalar2=None, op0=AluOp.mult,
                    )
                    return t

                wef0 = fold("wef_0", w_e[i][0], s_h)
                wef1b = fold("wef_1", w_e[i][1], s_h)
                wef2 = fold("wef_2", w_e[i][2], s_e)
                n1f0 = fold("n1f0", w_n1[i][0], s_h)
                n2f0 = fold("n2f0", w_n2[i][0], s_h)
                psb = ps_x.tile([P, P], F32, tag="mix", name="psb")
                nc.tensor.matmul(out=psb[:, 0:1], lhsT=w_e[i][0],
                                 rhs=t_h[:, 0:1], start=True, stop=False)
                nc.tensor.matmul(out=psb[:, 0:1], lhsT=w_e[i][1],
                                 rhs=t_h[:, 0:1], start=False, stop=False)
                nc.tensor.matmul(out=psb[:, 0:1], lhsT=w_e[i][2],
                                 rhs=t_e[:, 0:1], start=False, stop=True)
                bef = spool.tile([H, 1], F32, tag="bef", name="bef")
                nc.vector.tensor_tensor(
                    out=bef[:], in0=psb[:, 0:1], in1=be_col[i], op=AluOp.add
                )
                psb1 = ps_x.tile([P, P], F32, tag="mix", name="psb1")
                nc.tensor.matmul(out=psb1[:, 0:1], lhsT=w_n1[i][0],
                                 rhs=t_h[:, 0:1], start=True, stop=True)
                b1f = spool.tile([H, 1], F32, tag="b1f", name="b1f")
                nc.vector.tensor_tensor(
                    out=b1f[:], in0=psb1[:, 0:1], in1=b1_col[i], op=AluOp.add
                )
                psb2 = ps_x.tile([P, P], F32, tag="mix", name="psb2")
                nc.tensor.matmul(out=psb2[:, 0:1], lhsT=w_n2[i][0],
                                 rhs=t_h[:, 0:1], start=True, stop=True)
                b2f = spool.tile([H, 1], F32, tag="b2f", name="b2f")
                nc.vector.tensor_tensor(
                    out=b2f[:], in0=psb2[:, 0:1], in1=b2_col[i], op=AluOp.add
                )

                # broadcast b1f to [P, 512] for the edge-major psm add
                psr = ps_x.tile([P, P], F32, tag="mix", name="psr")
                nc.tensor.transpose(
                    out=psr[0:1, 0:P], in_=b1f[:, 0:1], identity=ident_f[:]
                )
                rowt4 = spool.tile([1, 512], F32, tag="b1_row", name="b1_row")
                for q in range(4):
                    copy_dve(rowt4[:, q * P:(q + 1) * P], psr[0:1, 0:P])
                psb_ = ps_m.tile([P, 512], F32, tag="psm", name="psb_")
                nc.tensor.matmul(
                    out=psb_[:], lhsT=ones_row[:], rhs=rowt4[:],
                    start=True, stop=True,
                )
                b1bc = spool.tile([P, 512], F32, tag="b1_bc", name="b1_bc")
                copy_dve(b1bc[:], psb_[:])

            # pad-slot e value for this layer (mirrors the stored bf16 chain)
            pspad = ps_x.tile([P, P], F32, tag="mix", name="pspad")
            nc.tensor.matmul(
                out=pspad[:, 0:1], lhsT=wef2[:], rhs=epad_bf[:, 0:1],
                start=True, stop=True,
            )
            epad_f = spool.tile([P, 1], F32, tag="epad_f", name="epad_f")
            nc.scalar.activation(
                out=epad_f[:], in_=pspad[:, 0:1], func=Act.Relu, bias=bef[:, 0:1]
            )
            epad_bf = spool.tile(
                [P, 1], BF16, tag=f"epad_bf_{'b' if i % 2 == 0 else 'a'}",
                name="epad_bf2",
            )
            nc.vector.tensor_copy(out=epad_bf[:], in_=epad_f[:])
            epx = spool.tile([P, 1], F32, tag="epx", name="epx")
            nc.vector.tensor_copy(out=epx[:], in_=epad_bf[:])
            # pad-node h value chain (agg contribution is 0 for pad nodes)
            pspadh = ps_x.tile([P, P], F32, tag="mix", name="pspadh")
            nc.tensor.matmul(
                out=pspadh[:, 0:1], lhsT=n2f0[:], rhs=hpad_bf[:, 0:1],
                start=True, stop=True,
            )
            hpad_f = spool.tile([P, 1], F32, tag="hpad_f", name="hpad_f")
            nc.scalar.activation(
                out=hpad_f[:], in_=pspadh[:, 0:1], func=Act.Relu, bias=b2f[:, 0:1]
            )
            hpad_bf = spool.tile(
                [P, 1], BF16, tag=f"hpad_bf_{'b' if i % 2 == 0 else 'a'}",
                name="hpad_bf2",
            )
            nc.vector.tensor_copy(out=hpad_bf[:], in_=hpad_f[:])
            hpx = spool.tile([P, 1], F32, tag="hpx", name="hpx")
            nc.vector.tensor_copy(out=hpx[:], in_=hpad_bf[:])

            # ---- stats accumulators ----
            se_cols = spool.tile([P, NW], F32, tag="se_cols", name="se_cols")
            se2_cols = spool.tile([P, NW], F32, tag="se2_cols", name="se2_cols")
            sh_cols = spool.tile([P, NG], F32, tag="sh_cols", name="sh_cols")
            sh2_cols = spool.tile([P, NG], F32, tag="sh2_cols", name="sh2_cols")

            # ---- window sweep ----
            off_lo = off_hi = 0
            for g in range(NG):
                w0 = g * GW
                gsz = min(GW, NW - w0)
                nlo, nhi = gsz * LOSL, gsz * HISL
                glo = gpool.tile([P, gsz * CLO, P], BF16, tag="glo", name="glo")
                ghi = gpool.tile([P, gsz * (CPW - CLO), P], BF16, tag="ghi",
                                 name="ghi")
                with tc.high_priority():
                    nc.gpsimd.dma_gather(
                        out_ap=glo[:], in_ap=htab[i][0:LO_LIM, :],
                        idxs_ap=ipr_lo_sb[:, off_lo:off_lo + nlo // 16],
                        num_idxs=nlo, num_idxs_reg=nlo, elem_size=H,
                        transpose=False, single_packet=False,
                        queue_num=(2 * g) % 4,
                    )
                    nc.gpsimd.dma_gather(
                        out_ap=ghi[:], in_ap=htab[i][HI_BASE:NPADT, :],
                        idxs_ap=ipr_hi_sb[:, off_hi:off_hi + nhi // 16],
                        num_idxs=nhi, num_idxs_reg=nhi, elem_size=H,
                        transpose=False, single_packet=False,
                        queue_num=(2 * g + 1) % 4,
                    )
                off_lo += nlo // 16
                off_hi += nhi // 16
                aggT_big = cpool.tile([P, 512], BF16, tag="aggT",
                                      name="aggT_big")
                for j in range(gsz):
                    w = w0 + j
                    ws, es = w * P, w * WSL
                    # feature-major h[row] for this window
                    hrT = wpool.tile([P, WSL], BF16, tag="hrT", name="hrT")
                    with tc.high_priority():
                        psT = ps_t.tile([P, 512], BF16, tag="psT", name="psT")
                        for ck in range(CPW):
                            src = (glo[:, j * CLO + ck, :] if ck < CLO
                                   else ghi[:, j * (CPW - CLO) + (ck - CLO), :])
                            nc.tensor.transpose(
                                out=psT[:, ck * P:(ck + 1) * P], in_=src,
                                identity=ident_bf[:],
                            )
                        copy_dve(hrT[:], psT[:])

                    ohne_w = wpool.tile([P, WSL], BF16, tag="ohne_w",
                                        name="ohne_w")
                    nc.sync.dma_start(out=ohne_w[:], in_=ohne_p[:, es:es + WSL])

                    # zT = (h_win)^T @ We1_folded   [node, H]
                    with tc.high_priority():
                        psz = ps_x.tile([P, P], F32, tag="mix", name="psz")
                        nc.tensor.matmul(
                            out=psz[:], lhsT=h_cur[:, ws:ws + P],
                            rhs=wef1b[:], start=True, stop=True,
                        )
                        zT = cpool.tile([P, P], BF16, tag="zT", name="zT")
                        copy_dve(zT[:], psz[:])

                    pse = ps_e.tile([P, 512], F32, tag="pse_w", name="pse")
                    nc.tensor.matmul(
                        out=pse[:], lhsT=wef0[:], rhs=hrT[:],
                        start=True, stop=False,
                    )
                    nc.tensor.matmul(
                        out=pse[:], lhsT=wef2[:], rhs=eT[:, es:es + WSL],
                        start=False, stop=False,
                    )
                    nc.tensor.matmul(
                        out=pse[:], lhsT=zT[:], rhs=ohne_w[:],
                        start=False, stop=True,
                    )
                    # in-place e update (WAR on the pse read is tracked)
                    nc.scalar.activation(
                        out=eT[:, es:es + WSL], in_=pse[:], func=Act.Relu,
                        bias=bef[:, 0:1], accum_out=se_cols[:, w:w + 1],
                    )
                    if not last:
                        dump_e = wpool.tile([P, WSL], BF16, tag="dump_e",
                                            name="dump_e")
                        nc.scalar.activation(
                            out=dump_e[:], in_=eT[:, es:es + WSL],
                            func=Act.Square, accum_out=se2_cols[:, w:w + 1],
                        )

                    psa = ps_a.tile([P, P], F32, tag="psa", name="psa")
                    psmb = ps_m.tile([P, 512], F32, tag="psm", name="psm")
                    for ck in range(CPW):
                        cs = es + ck * P
                        kg = w * CPW + ck
                        psm = psmb[:, ck * P:(ck + 1) * P]
                        nc.tensor.matmul(
                            out=psm, lhsT=hrT[:, ck * P:(ck + 1) * P],
                            rhs=n1f0[:], start=True, stop=False,
                        )
                        nc.tensor.matmul(
                            out=psm, lhsT=eT[:, cs:cs + P],
                            rhs=n1b1[i][:], start=False, stop=True,
                        )
                    if b1bc is not None:
                        nc.vector.tensor_tensor(
                            out=psmb[:], in0=psmb[:], in1=b1bc[:],
                            op=AluOp.add,
                        )
                    m_big = cpool.tile([P, 512], BF16, tag="m_sb",
                                       name="m_sb")
                    nc.scalar.activation(out=m_big[:], in_=psmb[:],
                                         func=Act.Relu)
                    for ck in range(CPW):
                        kg = w * CPW + ck
                        nc.tensor.matmul(
                            out=psa[:], lhsT=m_big[:, ck * P:(ck + 1) * P],
                            rhs=onehot_sb[:, kg * P:(kg + 1) * P],
                            start=(ck == 0), stop=(ck == CPW - 1),
                        )
                    nc.scalar.activation(
                        out=aggT_big[:, j * P:(j + 1) * P], in_=psa[:],
                        func=Act.Copy,
                    )

                # ---- group tail: n2 over gsz windows in two wide matmuls ----
                gws, gwid = w0 * P, gsz * P
                psf = ps_e.tile([P, 512], F32, tag="pse_w", name="psf")
                nc.tensor.matmul(
                    out=psf[:, :gwid], lhsT=n2f0[:],
                    rhs=h_cur[:, gws:gws + gwid], start=True, stop=False,
                )
                nc.tensor.matmul(
                    out=psf[:, :gwid], lhsT=wn2b1[i][:],
                    rhs=aggT_big[:, :gwid], start=False, stop=True,
                )
                nc.scalar.activation(
                    out=h_nxt[:, gws:gws + gwid], in_=psf[:, :gwid],
                    func=Act.Relu, bias=b2f[:, 0:1],
                    accum_out=sh_cols[:, g:g + 1],
                )
                if not last:
                    dump_h = cpool.tile([P, 512], BF16, tag="dump_h",
                                        name="dump_h")
                    nc.scalar.activation(
                        out=dump_h[:, :gwid], in_=h_nxt[:, gws:gws + gwid],
                        func=Act.Square, accum_out=sh2_cols[:, g:g + 1],
                    )
                    # node-major windows for the table; pad-node rows are
                    # never gathered, so no masking is needed
                    psh = ps_t.tile([P, 512], BF16, tag="psT", name="psh")
                    for j in range(gsz):
                        ws = (w0 + j) * P
                        nc.tensor.transpose(
                            out=psh[:, j * P:(j + 1) * P],
                            in_=h_nxt[:, ws:ws + P], identity=ident_bf[:],
                        )
                    hwin_bf = cpool.tile([P, 512], BF16, tag="hwin_bf",
                                         name="hwin_bf")
                    nc.scalar.activation(
                        out=hwin_bf[:, :gwid], in_=psh[:, :gwid], func=Act.Copy
                    )
                    nc.sync.dma_start(
                        out=hsegR[i + 1][:, w0:w0 + gsz, :],
                        in_=hwin_bf[:, :gwid],
                    )

            # ---- end of layer: tiny stats AllReduce first, so the BN
            # folds (which wait on it) overlap the big AllGather ----
            if not last:
                nc.sync.dma_start(out=hseg[i + 1][NPC:NPCT, :], in_=zero_bf[:])

            # ---- stats AllReduce ----
            ar_sb = spool.tile([P, 4], F32, tag="ar_sb", name="ar_sb")

            def stat_col(dst, cols, pad_val, pad_cnt):
                nc.vector.tensor_reduce(
                    out=dst, in_=cols[:], axis=mybir.AxisListType.X,
                    op=AluOp.add,
                )
                cor = spool.tile([P, 1], F32, tag="cor", name="cor")
                nc.vector.tensor_tensor(
                    out=cor[:], in0=pad_val, in1=pad_cnt, op=AluOp.mult
                )
                nc.vector.tensor_tensor(
                    out=dst, in0=dst, in1=cor[:], op=AluOp.subtract
                )

            # sums read the stored bf16 values -> corrections use the
            # bf16-rounded pad values (epx/hpx)
            stat_col(ar_sb[:, 0:1], se_cols, epx[:], npads_sb[:])
            stat_col(ar_sb[:, 2:3], sh_cols, hpx[:], npadn_sb[:])
            if not last:
                esq = spool.tile([P, 1], F32, tag="esq", name="esq")
                nc.vector.tensor_tensor(
                    out=esq[:], in0=epx[:], in1=epx[:], op=AluOp.mult
                )
                stat_col(ar_sb[:, 1:2], se2_cols, esq[:], npads_sb[:])
                hsq = spool.tile([P, 1], F32, tag="hsq", name="hsq")
                nc.vector.tensor_tensor(
                    out=hsq[:], in0=hpx[:], in1=hpx[:], op=AluOp.mult
                )
                stat_col(ar_sb[:, 3:4], sh2_cols, hsq[:], npadn_sb[:])
            else:
                nc.vector.memset(ar_sb[:, 1:2], 0.0)
                nc.vector.memset(ar_sb[:, 3:4], 0.0)
            nc.sync.dma_start(out=ar_in[i][:, :], in_=ar_sb[:])
            nc.gpsimd.collective_compute(
                "AllReduce", AluOp.add, replica_groups=rg,
                ins=[ar_in[i][:, :]], outs=[ar_out[i][:, :]],
            )
            if not last:
                nc.gpsimd.collective_compute(
                    "AllGather", AluOp.bypass, replica_groups=rg,
                    ins=[hseg[i + 1][:, :]], outs=[htab[i + 1][:, :]],
                )
            arr = spool.tile([P, 4], F32, tag="arr", name="arr")
            nc.sync.dma_start(out=arr[:], in_=ar_out[i][:, :])

            if not last:
                def bn_consts(sum_c, sq_c, inv_cnt, g_t, b_t, tag):
                    mean = spool.tile([P, 1], F32, tag=f"mean_{tag}",
                                      name=f"mean_{tag}")
                    nc.vector.tensor_scalar(
                        out=mean[:], in0=sum_c, scalar1=inv_cnt, scalar2=None,
                        op0=AluOp.mult,
                    )
                    var = spool.tile([P, 1], F32, tag=f"var_{tag}",
                                     name=f"var_{tag}")
                    nc.vector.tensor_scalar(
                        out=var[:], in0=sq_c, scalar1=inv_cnt, scalar2=None,
                        op0=AluOp.mult,
                    )
                    m2 = spool.tile([P, 1], F32, tag=f"m2_{tag}",
                                    name=f"m2_{tag}")
                    nc.vector.tensor_tensor(
                        out=m2[:], in0=mean[:], in1=mean[:], op=AluOp.mult
                    )
                    nc.vector.tensor_tensor(
                        out=var[:], in0=var[:], in1=m2[:], op=AluOp.subtract
                    )
                    sd = spool.tile([P, 1], F32, tag=f"sd_{tag}",
                                    name=f"sd_{tag}")
                    nc.scalar.activation(
                        out=sd[:], in_=var[:], func=Act.Sqrt,
                        bias=eps_sb[:, 0:1],
                    )
                    rs = spool.tile([P, 1], F32, tag=f"rs_{tag}",
                                    name=f"rs_{tag}")
                    nc.vector.reciprocal(out=rs[:], in_=sd[:])
                    s = spool.tile([P, 1], F32, tag=f"s_{tag}",
                                   name=f"s_{tag}")
                    nc.vector.tensor_tensor(
                        out=s[:], in0=rs[:], in1=g_t[:], op=AluOp.mult
                    )
                    ms = spool.tile([P, 1], F32, tag=f"ms_{tag}",
                                    name=f"ms_{tag}")
                    nc.vector.tensor_tensor(
                        out=ms[:], in0=mean[:], in1=s[:], op=AluOp.mult
                    )
                    t = spool.tile([P, 1], F32, tag=f"t_{tag}",
                                   name=f"t_{tag}")
                    nc.vector.tensor_tensor(
                        out=t[:], in0=b_t[:], in1=ms[:], op=AluOp.subtract
                    )
                    return s, t

                s_e, t_e = bn_consts(arr[:, 0:1], arr[:, 1:2], inv_e, beg,
                                     beb, "e")
                s_h, t_h = bn_consts(arr[:, 2:3], arr[:, 3:4], inv_n, bng,
                                     bnb, "h")
            else:
                roh = spool.tile([P, 1], F32, tag="roh", name="roh")
                nc.vector.tensor_scalar(
                    out=roh[:], in0=arr[:, 2:3], scalar1=inv_n, scalar2=None,
                    op0=AluOp.mult,
                )
                roe = spool.tile([P, 1], F32, tag="roe", name="roe")
                nc.vector.tensor_scalar(
                    out=roe[:], in0=arr[:, 0:1], scalar1=inv_e, scalar2=None,
                    op0=AluOp.mult,
                )
                pso = ps_x.tile([P, P], F32, tag="mix", name="pso")
                nc.tensor.matmul(
                    out=pso[0:1, 0:1], lhsT=roh[:, 0:1], rhs=regw_h[:, 0:1],
                    start=True, stop=False,
                )
                nc.tensor.matmul(
                    out=pso[0:1, 0:1], lhsT=roe[:, 0:1], rhs=regw_e[:, 0:1],
                    start=False, stop=True,
                )
                out_sb = spool.tile([1, 1], F32, tag="out_sb", name="out_sb")
                nc.vector.tensor_tensor(
                    out=out_sb[:], in0=pso[0:1, 0:1], in1=regb_sb[:],
                    op=AluOp.add,
                )
                nc.sync.dma_start(out=out_p[:, :], in_=out_sb[:])

    nc.compile()
    return nc


def kernel(**inputs) -> np.ndarray:
    cfg = Cfg()
    maps = prep(cfg, inputs["x"], inputs["edge_index"], inputs["edge_attr"])
    add_weight_params(cfg, maps, inputs)
    nc = build(cfg)
    from concourse.bass_utils import run_bass_kernel_spmd

    res = run_bass_kernel_spmd(nc, maps, list(range(cfg.NC)))
    return np.asarray(res.results[0]["out"], np.float32)
